# revision 17
# baseline (speedup 1.0000x reference)
"""Trainium2 Bass kernel for CubicShapeFunction (500k particles, 4^3 cubic
B-spline stencil), data-parallel over 8 NeuronCores.

Math: rel = pos*20, f = frac(rel) in [0,1), g = 1-f. The stencil distance
for offset o in {0,1,2,3} is f+1-o, which falls in spline branch
c4,c3,c2,c1 respectively, so (h = 20):

  B0 = g^3/6              DB0 = -h g^2/2
  B1 = 2/3 - f^2(1-f/2)   DB1 = 1.5h (f-2/3)^2 - 2h/3
  B2 = 2/3 - g^2(1-g/2)   DB2 = -1.5h (g-2/3)^2 + 2h/3
  B3 = f^3/6              DB3 = h f^2/2

outputs:
  shapef[n, k]  = B0[oi]*B1[oj]*B2[ok],        k = oi*16+oj*4+ok
  grad[n, k, d] = DBd[od] * prod_{d'!=d} Bd'[od']

Structure: the cheap scalar "basis stage" (frac, squares, all 24 basis
values) is batched over CHUNK=4 tiles at a time to amortize per-
instruction fixed costs and cross-engine semaphores; the bandwidth-bound
outer-product stage runs per tile (128 partitions x fp=42 particles):

  Scalar(ACT): affine/square ops for all basis columns (own SBUF ports)
  Vector(DVE): exact floor trick, B0/B3 cubes (fused STT), and the four
               big (pair[ij] x z-basis[k]) products
  GpSimd:      frac fixup add, B1/B2 muls, pair products P01/R01/R11
All output tiles mirror DRAM layout, so S and G DMAs are contiguous.
"""

import numpy as np

import concourse.bass as bass  # noqa: F401
import concourse.bacc as bacc
import concourse.tile as tile
from concourse import mybir
from concourse import bass_utils

F32 = mybir.dt.float32
H = 20.0
MAGIC = float(2 ** 23)

N_TOTAL = 500_000
N_CORES = 8
N_PER_CORE = N_TOTAL // N_CORES          # 62500
FP = 42                                   # particles per partition per tile
NTILES = 12
CHUNK = 4                                 # tiles per basis-stage batch
NPC = 128 * FP * NTILES                   # 64512 padded rows per core


def build_module(n_rows: int = NPC, fp: int = FP, chunk: int = CHUNK,
                 num_devices: int = N_CORES, enable_asserts: bool = False):
    P = 128
    rows_per_tile = P * fp
    assert n_rows % (rows_per_tile * chunk) == 0
    ntiles = n_rows // rows_per_tile
    nchunks = ntiles // chunk
    cf = chunk * fp  # particles per partition per chunk

    nc = bacc.Bacc(
        "TRN2",
        target_bir_lowering=False,
        debug=False,
        enable_asserts=enable_asserts,
        num_devices=num_devices,
    )
    # pre-register activation-bias constants (only 0.0/1.0 exist by default)
    for value in (2.0 / 3.0, -2.0 / 3.0, -2.0 * H / 3.0, 2.0 * H / 3.0):
        t = nc.alloc_sbuf_tensor(f"const-f32-{value}", [128, 1], F32)
        nc.gpsimd.memset(t.ap(), value)
        nc.const_aps.aps[(F32, value)] = t.ap()
    nc.all_engine_barrier()

    pos_d = nc.dram_tensor("pos", [n_rows, 3], F32, kind="ExternalInput").ap()
    shapef_d = nc.dram_tensor("shapef", [n_rows, 64], F32, kind="ExternalOutput").ap()
    grad_d = nc.dram_tensor("grad", [n_rows, 192], F32, kind="ExternalOutput").ap()

    pos_v = pos_d.rearrange("(t p q) d -> p t (q d)", t=ntiles, p=P)
    sh_v = shapef_d.rearrange("(t p q) w -> t p (q w)", t=ntiles, p=P)
    gr_v = grad_d.rearrange("(t p q) w -> t p (q w)", t=ntiles, p=P)

    A = mybir.AluOpType
    ID = mybir.ActivationFunctionType.Identity
    SQ = mybir.ActivationFunctionType.Square

    with tile.TileContext(nc) as tc:
        with (
            tc.tile_pool(name="const", bufs=1) as constp,
            tc.tile_pool(name="tmp", bufs=1) as tmp,
            tc.tile_pool(name="bdb", bufs=2) as bdbp,
            tc.tile_pool(name="pp", bufs=2) as ppp,
            tc.tile_pool(name="sg", bufs=2) as sgp,
        ):
            POS = constp.tile([P, ntiles, fp * 3], F32)
            nc.sync.dma_start(out=POS, in_=pos_v)

            for c in range(nchunks):
                # ---- basis stage for `chunk` tiles at once ----
                posc = POS[:, c * chunk:(c + 1) * chunk, :].rearrange(
                    "p t (q d) -> p (t q) d", d=3)

                def t3(tag):
                    return tmp.tile([P, cf, 3], F32, tag=tag, name=tag)

                rel, fl, gt, fr, g = t3("rel"), t3("fl"), t3("gt"), t3("fr"), t3("g")
                f2, g2 = t3("f2"), t3("g2")
                t1, t2, w1, w2 = t3("t1"), t3("t2"), t3("w1"), t3("w2")
                b0t, b3t = t3("b0t"), t3("b3t")

                # rel = 20*pos; exact floor: round via +-2^23, minus (round>rel)
                nc.scalar.mul(out=rel, in_=posc, mul=H)
                nc.vector.tensor_scalar(out=fl, in0=rel, scalar1=MAGIC, scalar2=MAGIC,
                                        op0=A.add, op1=A.subtract)
                nc.vector.tensor_tensor(out=gt, in0=fl, in1=rel, op=A.is_gt)
                nc.vector.tensor_tensor(out=fr, in0=rel, in1=fl, op=A.subtract)
                nc.gpsimd.tensor_tensor(out=fr, in0=fr, in1=gt, op=A.add)
                nc.scalar.activation(out=g, in_=fr, func=ID, bias=1.0, scale=-1.0)

                nc.scalar.activation(out=f2, in_=fr, func=SQ)
                nc.scalar.activation(out=g2, in_=g, func=SQ)

                B = bdbp.tile([P, chunk, fp, 3, 4], F32, tag="B")
                DB = bdbp.tile([P, chunk, fp, 3, 4], F32, tag="DB")

                def col(T, o):
                    return T[:, :, :, :, o]

                def cview(x):  # [P, cf, 3] -> [P, chunk, fp, 3]
                    return x.rearrange("p (t q) d -> p t q d", t=chunk)

                # B0 = g^3/6, B3 = f^3/6 via fused (sq*1/6)*lin into contiguous
                # temps; ACT copies into strided columns.
                nc.vector.scalar_tensor_tensor(out=b0t, in0=g2, scalar=1.0 / 6.0,
                                               in1=g, op0=A.mult, op1=A.mult)
                nc.vector.scalar_tensor_tensor(out=b3t, in0=f2, scalar=1.0 / 6.0,
                                               in1=fr, op0=A.mult, op1=A.mult)
                nc.scalar.copy(out=col(B, 0), in_=cview(b0t))
                nc.scalar.copy(out=col(B, 3), in_=cview(b3t))
                nc.scalar.activation(out=t1, in_=fr, func=ID, bias=1.0, scale=-0.5)
                nc.gpsimd.tensor_tensor(out=t1, in0=f2, in1=t1, op=A.mult)
                nc.scalar.activation(out=col(B, 1), in_=cview(t1), func=ID,
                                     bias=2.0 / 3.0, scale=-1.0)
                nc.scalar.activation(out=t2, in_=g, func=ID, bias=1.0, scale=-0.5)
                nc.gpsimd.tensor_tensor(out=t2, in0=g2, in1=t2, op=A.mult)
                nc.scalar.activation(out=col(B, 2), in_=cview(t2), func=ID,
                                     bias=2.0 / 3.0, scale=-1.0)

                # dbasis columns, pure ACT (square-completion for DB1/DB2)
                nc.scalar.mul(out=col(DB, 0), in_=cview(g2), mul=-0.5 * H)
                nc.scalar.mul(out=col(DB, 3), in_=cview(f2), mul=0.5 * H)
                nc.scalar.activation(out=w1, in_=fr, func=SQ, bias=-2.0 / 3.0, scale=1.0)
                nc.scalar.activation(out=col(DB, 1), in_=cview(w1), func=ID,
                                     bias=-2.0 * H / 3.0, scale=1.5 * H)
                nc.scalar.activation(out=w2, in_=g, func=SQ, bias=-2.0 / 3.0, scale=1.0)
                nc.scalar.activation(out=col(DB, 2), in_=cview(w2), func=ID,
                                     bias=2.0 * H / 3.0, scale=-1.5 * H)

                # ---- per-tile product stage ----
                for s in range(chunk):
                    it = c * chunk + s

                    def bx(d):
                        return B[:, s, :, d, :]

                    def dbx(d):
                        return DB[:, s, :, d, :]

                    P01 = ppp.tile([P, fp, 4, 4], F32, tag="P01")
                    R01 = ppp.tile([P, fp, 4, 4], F32, tag="R01")
                    R11 = ppp.tile([P, fp, 4, 4], F32, tag="R11")
                    nc.gpsimd.tensor_tensor(
                        out=P01,
                        in0=bx(0).unsqueeze(3).broadcast_to([P, fp, 4, 4]),
                        in1=bx(1).unsqueeze(2).broadcast_to([P, fp, 4, 4]),
                        op=A.mult)
                    nc.gpsimd.tensor_tensor(
                        out=R01,
                        in0=dbx(0).unsqueeze(3).broadcast_to([P, fp, 4, 4]),
                        in1=bx(1).unsqueeze(2).broadcast_to([P, fp, 4, 4]),
                        op=A.mult)
                    nc.gpsimd.tensor_tensor(
                        out=R11,
                        in0=bx(0).unsqueeze(3).broadcast_to([P, fp, 4, 4]),
                        in1=dbx(1).unsqueeze(2).broadcast_to([P, fp, 4, 4]),
                        op=A.mult)

                    S = sgp.tile([P, fp, 16, 4], F32, tag="S")
                    p01f = P01.rearrange("p q i j -> p q (i j)")
                    r01f = R01.rearrange("p q i j -> p q (i j)")
                    r11f = R11.rearrange("p q i j -> p q (i j)")
                    nc.vector.tensor_tensor(
                        out=S,
                        in0=p01f.unsqueeze(3).broadcast_to([P, fp, 16, 4]),
                        in1=bx(2).unsqueeze(2).broadcast_to([P, fp, 16, 4]),
                        op=A.mult)

                    G = sgp.tile([P, fp, 64, 3], F32, tag="G")
                    gm = G.rearrange("p q (m k) d -> p q m k d", m=16)
                    nc.vector.tensor_tensor(
                        out=gm[:, :, :, :, 0],
                        in0=r01f.unsqueeze(3).broadcast_to([P, fp, 16, 4]),
                        in1=bx(2).unsqueeze(2).broadcast_to([P, fp, 16, 4]),
                        op=A.mult)
                    nc.vector.tensor_tensor(
                        out=gm[:, :, :, :, 1],
                        in0=r11f.unsqueeze(3).broadcast_to([P, fp, 16, 4]),
                        in1=bx(2).unsqueeze(2).broadcast_to([P, fp, 16, 4]),
                        op=A.mult)
                    nc.vector.tensor_tensor(
                        out=gm[:, :, :, :, 2],
                        in0=p01f.unsqueeze(3).broadcast_to([P, fp, 16, 4]),
                        in1=dbx(2).unsqueeze(2).broadcast_to([P, fp, 16, 4]),
                        op=A.mult)

                    nc.sync.dma_start(out=sh_v[it], in_=S.rearrange("p q m k -> p (q m k)"))
                    nc.sync.dma_start(out=gr_v[it], in_=G.rearrange("p q w d -> p (q w d)"))

    nc.compile()
    return nc


_MODULE_CACHE = {}


def _get_module():
    if "nc" not in _MODULE_CACHE:
        _MODULE_CACHE["nc"] = build_module()
    return _MODULE_CACHE["nc"]


def run(position_stack: np.ndarray, trace: bool = False):
    """Run on 8 NeuronCores. Returns ((shapef, grad), BassKernelResults)."""
    pos = np.ascontiguousarray(np.asarray(position_stack, dtype=np.float32))
    assert pos.shape == (N_TOTAL, 3), pos.shape

    nc = _get_module()
    in_maps = []
    for c in range(N_CORES):
        sl = pos[c * N_PER_CORE : (c + 1) * N_PER_CORE]
        padded = np.full((NPC, 3), 0.5, dtype=np.float32)
        padded[:N_PER_CORE] = sl
        in_maps.append({"pos": padded})

    res = bass_utils.run_bass_kernel_spmd(
        nc, in_maps, core_ids=list(range(N_CORES)), trace=trace,
    )

    shapef = np.empty((N_TOTAL, 64), dtype=np.float32)
    grad = np.empty((N_TOTAL, 64, 3), dtype=np.float32)
    for c, r in enumerate(res.results):
        shapef[c * N_PER_CORE : (c + 1) * N_PER_CORE] = r["shapef"][:N_PER_CORE]
        grad[c * N_PER_CORE : (c + 1) * N_PER_CORE] = r["grad"][:N_PER_CORE].reshape(
            N_PER_CORE, 64, 3
        )
    return (shapef, grad), res


def kernel(position_stack: np.ndarray):
    (shapef, grad), _ = run(position_stack, trace=False)
    return shapef, grad


# revision 18
# speedup vs baseline: 1.0227x; 1.0227x over previous
"""Trainium2 Bass kernel for CubicShapeFunction (500k particles, 4^3 cubic
B-spline stencil), data-parallel over 8 NeuronCores.

Math: rel = pos*20, f = frac(rel) in [0,1), g = 1-f. The stencil distance
for offset o in {0,1,2,3} is f+1-o, which falls in spline branch
c4,c3,c2,c1 respectively, so (h = 20):

  B0 = g^3/6              DB0 = -h g^2/2
  B1 = 2/3 - f^2(1-f/2)   DB1 = 1.5h (f-2/3)^2 - 2h/3
  B2 = 2/3 - g^2(1-g/2)   DB2 = -1.5h (g-2/3)^2 + 2h/3
  B3 = f^3/6              DB3 = h f^2/2

outputs:
  shapef[n, k]  = B0[oi]*B1[oj]*B2[ok],        k = oi*16+oj*4+ok
  grad[n, k, d] = DBd[od] * prod_{d'!=d} Bd'[od']

Structure: the cheap scalar "basis stage" (frac, squares, all 24 basis
values) is batched over CHUNK=4 tiles at a time to amortize per-
instruction fixed costs and cross-engine semaphores; the bandwidth-bound
outer-product stage runs per tile (128 partitions x fp=42 particles):

  Scalar(ACT): affine/square ops for all basis columns (own SBUF ports)
  Vector(DVE): exact floor trick, B0/B3 cubes (fused STT), and the four
               big (pair[ij] x z-basis[k]) products
  GpSimd:      frac fixup add, B1/B2 muls, pair products P01/R01/R11
All output tiles mirror DRAM layout, so S and G DMAs are contiguous.
"""

import numpy as np

import concourse.bass as bass  # noqa: F401
import concourse.bacc as bacc
import concourse.tile as tile
from concourse import mybir
from concourse import bass_utils

F32 = mybir.dt.float32
H = 20.0
MAGIC = float(2 ** 23)

N_TOTAL = 500_000
N_CORES = 8
N_PER_CORE = N_TOTAL // N_CORES          # 62500
FP = 42                                   # particles per partition per tile
NTILES = 12
CHUNK = 4                                 # tiles per basis-stage batch
NPC = 128 * FP * NTILES                   # 64512 padded rows per core


def build_module(n_rows: int = NPC, fp: int = FP, chunk: int = CHUNK,
                 num_devices: int = N_CORES, enable_asserts: bool = False):
    P = 128
    rows_per_tile = P * fp
    assert n_rows % (rows_per_tile * chunk) == 0
    ntiles = n_rows // rows_per_tile
    nchunks = ntiles // chunk
    cf = chunk * fp  # particles per partition per chunk

    nc = bacc.Bacc(
        "TRN2",
        target_bir_lowering=False,
        debug=False,
        enable_asserts=enable_asserts,
        num_devices=num_devices,
    )
    # pre-register activation-bias constants (only 0.0/1.0 exist by default)
    for value in (2.0 / 3.0, -2.0 / 3.0, -2.0 * H / 3.0, 2.0 * H / 3.0):
        t = nc.alloc_sbuf_tensor(f"const-f32-{value}", [128, 1], F32)
        nc.gpsimd.memset(t.ap(), value)
        nc.const_aps.aps[(F32, value)] = t.ap()
    nc.all_engine_barrier()

    pos_d = nc.dram_tensor("pos", [n_rows, 3], F32, kind="ExternalInput").ap()
    shapef_d = nc.dram_tensor("shapef", [n_rows, 64], F32, kind="ExternalOutput").ap()
    grad_d = nc.dram_tensor("grad", [n_rows, 192], F32, kind="ExternalOutput").ap()

    pos_v = pos_d.rearrange("(t p q) d -> p t (q d)", t=ntiles, p=P)
    sh_v = shapef_d.rearrange("(t p q) w -> t p (q w)", t=ntiles, p=P)
    gr_v = grad_d.rearrange("(t p q) w -> t p (q w)", t=ntiles, p=P)

    A = mybir.AluOpType
    ID = mybir.ActivationFunctionType.Identity
    SQ = mybir.ActivationFunctionType.Square

    with tile.TileContext(nc) as tc:
        with (
            tc.tile_pool(name="const", bufs=1) as constp,
            tc.tile_pool(name="tmp", bufs=1) as tmp,
            tc.tile_pool(name="bdb", bufs=2) as bdbp,
            tc.tile_pool(name="pp", bufs=3) as ppp,
            tc.tile_pool(name="sg", bufs=2) as sgp,
        ):
            POS = constp.tile([P, ntiles, fp * 3], F32)
            nc.sync.dma_start(out=POS, in_=pos_v)

            for c in range(nchunks):
                # ---- basis stage for `chunk` tiles at once ----
                posc = POS[:, c * chunk:(c + 1) * chunk, :].rearrange(
                    "p t (q d) -> p (t q) d", d=3)

                def t3(tag):
                    return tmp.tile([P, cf, 3], F32, tag=tag, name=tag)

                rel, fl, gt, fr, g = t3("rel"), t3("fl"), t3("gt"), t3("fr"), t3("g")
                f2, g2 = t3("f2"), t3("g2")
                t1, t2, w1, w2 = t3("t1"), t3("t2"), t3("w1"), t3("w2")
                b0t, b3t = t3("b0t"), t3("b3t")

                # rel = 20*pos; exact floor: round via +-2^23, minus (round>rel)
                nc.scalar.mul(out=rel, in_=posc, mul=H)
                nc.vector.tensor_scalar(out=fl, in0=rel, scalar1=MAGIC, scalar2=MAGIC,
                                        op0=A.add, op1=A.subtract)
                nc.vector.tensor_tensor(out=gt, in0=fl, in1=rel, op=A.is_gt)
                nc.vector.tensor_tensor(out=fr, in0=rel, in1=fl, op=A.subtract)
                nc.gpsimd.tensor_tensor(out=fr, in0=fr, in1=gt, op=A.add)
                nc.scalar.activation(out=g, in_=fr, func=ID, bias=1.0, scale=-1.0)

                nc.scalar.activation(out=f2, in_=fr, func=SQ)
                nc.scalar.activation(out=g2, in_=g, func=SQ)

                B = bdbp.tile([P, chunk, fp, 3, 4], F32, tag="B")
                DB = bdbp.tile([P, chunk, fp, 3, 4], F32, tag="DB")

                def col(T, o):
                    return T[:, :, :, :, o]

                def cview(x):  # [P, cf, 3] -> [P, chunk, fp, 3]
                    return x.rearrange("p (t q) d -> p t q d", t=chunk)

                # B0 = g^3/6, B3 = f^3/6 via fused (sq*1/6)*lin into contiguous
                # temps; ACT copies into strided columns.
                nc.vector.scalar_tensor_tensor(out=b0t, in0=g2, scalar=1.0 / 6.0,
                                               in1=g, op0=A.mult, op1=A.mult)
                nc.vector.scalar_tensor_tensor(out=b3t, in0=f2, scalar=1.0 / 6.0,
                                               in1=fr, op0=A.mult, op1=A.mult)
                nc.scalar.copy(out=col(B, 0), in_=cview(b0t))
                nc.scalar.copy(out=col(B, 3), in_=cview(b3t))
                nc.scalar.activation(out=t1, in_=fr, func=ID, bias=1.0, scale=-0.5)
                nc.gpsimd.tensor_tensor(out=t1, in0=f2, in1=t1, op=A.mult)
                nc.scalar.activation(out=col(B, 1), in_=cview(t1), func=ID,
                                     bias=2.0 / 3.0, scale=-1.0)
                nc.scalar.activation(out=t2, in_=g, func=ID, bias=1.0, scale=-0.5)
                nc.gpsimd.tensor_tensor(out=t2, in0=g2, in1=t2, op=A.mult)
                nc.scalar.activation(out=col(B, 2), in_=cview(t2), func=ID,
                                     bias=2.0 / 3.0, scale=-1.0)

                # dbasis columns, pure ACT (square-completion for DB1/DB2)
                nc.scalar.mul(out=col(DB, 0), in_=cview(g2), mul=-0.5 * H)
                nc.scalar.mul(out=col(DB, 3), in_=cview(f2), mul=0.5 * H)
                nc.scalar.activation(out=w1, in_=fr, func=SQ, bias=-2.0 / 3.0, scale=1.0)
                nc.scalar.activation(out=col(DB, 1), in_=cview(w1), func=ID,
                                     bias=-2.0 * H / 3.0, scale=1.5 * H)
                nc.scalar.activation(out=w2, in_=g, func=SQ, bias=-2.0 / 3.0, scale=1.0)
                nc.scalar.activation(out=col(DB, 2), in_=cview(w2), func=ID,
                                     bias=2.0 * H / 3.0, scale=-1.5 * H)

                # ---- per-tile product stage ----
                for s in range(chunk):
                    it = c * chunk + s

                    def bx(d):
                        return B[:, s, :, d, :]

                    def dbx(d):
                        return DB[:, s, :, d, :]

                    P01 = ppp.tile([P, fp, 4, 4], F32, tag="P01")
                    R01 = ppp.tile([P, fp, 4, 4], F32, tag="R01")
                    R11 = ppp.tile([P, fp, 4, 4], F32, tag="R11")
                    nc.gpsimd.tensor_tensor(
                        out=P01,
                        in0=bx(0).unsqueeze(3).broadcast_to([P, fp, 4, 4]),
                        in1=bx(1).unsqueeze(2).broadcast_to([P, fp, 4, 4]),
                        op=A.mult)
                    nc.gpsimd.tensor_tensor(
                        out=R01,
                        in0=dbx(0).unsqueeze(3).broadcast_to([P, fp, 4, 4]),
                        in1=bx(1).unsqueeze(2).broadcast_to([P, fp, 4, 4]),
                        op=A.mult)
                    nc.gpsimd.tensor_tensor(
                        out=R11,
                        in0=bx(0).unsqueeze(3).broadcast_to([P, fp, 4, 4]),
                        in1=dbx(1).unsqueeze(2).broadcast_to([P, fp, 4, 4]),
                        op=A.mult)

                    S = sgp.tile([P, fp, 16, 4], F32, tag="S")
                    p01f = P01.rearrange("p q i j -> p q (i j)")
                    r01f = R01.rearrange("p q i j -> p q (i j)")
                    r11f = R11.rearrange("p q i j -> p q (i j)")
                    nc.vector.tensor_tensor(
                        out=S,
                        in0=p01f.unsqueeze(3).broadcast_to([P, fp, 16, 4]),
                        in1=bx(2).unsqueeze(2).broadcast_to([P, fp, 16, 4]),
                        op=A.mult)

                    G = sgp.tile([P, fp, 64, 3], F32, tag="G")
                    gm = G.rearrange("p q (m k) d -> p q m k d", m=16)
                    nc.vector.tensor_tensor(
                        out=gm[:, :, :, :, 0],
                        in0=r01f.unsqueeze(3).broadcast_to([P, fp, 16, 4]),
                        in1=bx(2).unsqueeze(2).broadcast_to([P, fp, 16, 4]),
                        op=A.mult)
                    nc.vector.tensor_tensor(
                        out=gm[:, :, :, :, 1],
                        in0=r11f.unsqueeze(3).broadcast_to([P, fp, 16, 4]),
                        in1=bx(2).unsqueeze(2).broadcast_to([P, fp, 16, 4]),
                        op=A.mult)
                    nc.vector.tensor_tensor(
                        out=gm[:, :, :, :, 2],
                        in0=p01f.unsqueeze(3).broadcast_to([P, fp, 16, 4]),
                        in1=dbx(2).unsqueeze(2).broadcast_to([P, fp, 16, 4]),
                        op=A.mult)

                    nc.sync.dma_start(out=sh_v[it], in_=S.rearrange("p q m k -> p (q m k)"))
                    nc.sync.dma_start(out=gr_v[it], in_=G.rearrange("p q w d -> p (q w d)"))

    nc.compile()
    return nc


_MODULE_CACHE = {}


def _get_module():
    if "nc" not in _MODULE_CACHE:
        _MODULE_CACHE["nc"] = build_module()
    return _MODULE_CACHE["nc"]


def run(position_stack: np.ndarray, trace: bool = False):
    """Run on 8 NeuronCores. Returns ((shapef, grad), BassKernelResults)."""
    pos = np.ascontiguousarray(np.asarray(position_stack, dtype=np.float32))
    assert pos.shape == (N_TOTAL, 3), pos.shape

    nc = _get_module()
    in_maps = []
    for c in range(N_CORES):
        sl = pos[c * N_PER_CORE : (c + 1) * N_PER_CORE]
        padded = np.full((NPC, 3), 0.5, dtype=np.float32)
        padded[:N_PER_CORE] = sl
        in_maps.append({"pos": padded})

    res = bass_utils.run_bass_kernel_spmd(
        nc, in_maps, core_ids=list(range(N_CORES)), trace=trace,
    )

    shapef = np.empty((N_TOTAL, 64), dtype=np.float32)
    grad = np.empty((N_TOTAL, 64, 3), dtype=np.float32)
    for c, r in enumerate(res.results):
        shapef[c * N_PER_CORE : (c + 1) * N_PER_CORE] = r["shapef"][:N_PER_CORE]
        grad[c * N_PER_CORE : (c + 1) * N_PER_CORE] = r["grad"][:N_PER_CORE].reshape(
            N_PER_CORE, 64, 3
        )
    return (shapef, grad), res


def kernel(position_stack: np.ndarray):
    (shapef, grad), _ = run(position_stack, trace=False)
    return shapef, grad
